# revision 3
# baseline (speedup 1.0000x reference)
"""Trainium2 Bass kernel for nn_FCond (FiLM-conditioned MLP chain).

Reference computation (B=32, N=100000, D=3, CDIM=128):
    h = x
    for kblk in [0, 1, 2, 2, 2, 2]:
        h = tanh((h @ Wk.T + bk) * sigmoid(c @ Wsk.T + bsk) + (c @ Wbk.T + bbk))

Since the FiLM conditioning depends only on (c, weights), each (batch,
block) reduces to an affine map  h' = tanh(A_kb @ h + d_kb)  with
A_kb [3,3], d_kb [3] precomputed on the host in float64.

Device strategy (pure data parallel over 8 cores, 4 batches/core):
  - Layout: partition p = b*32 + comp*10 + g  (4 batch-bands of 32
    partitions; 3 comps x 10 point-groups per band; rows 30,31 of each
    band are zero padding). Free dim = 10000 points per (b,comp,g)
    stream.
  - Each block is ONE block-diagonal [128x128] fp16 matmul on TensorE
    (40 real points per column, 1 cyc/col), PSUM f32 accumulated, then
    ScalarE does tanh(psum + d) with a per-partition bias AP,
    evacuating PSUM->SBUF as fp16.
  - Hand-scheduled engine programs (no TileContext): 36 stages
    s=(kblk, chunk) over 6 column chunks, rotating in two groups of 3
    chunks so each stage's input (written 3 stages earlier by ACT) and
    the PSUM ping-pong WAR (2 stages earlier) are both satisfied by a
    single act_sem >= s-1 wait. ACT (tanh, 1.2 GHz, 1 elem/cyc/lane)
    is the roofline engine; PE at fp16 has ~2x headroom so no warmup
    burst is needed. Input x chunks stream on the sync queue, weights/
    bias on the gpsimd queue, outputs DMA out per-chunk as soon as the
    last block's tanh lands (final chunk split in half to overlap the
    tail DMA).

Numerics: weights/activations fp16 (PE @ 1 cyc/col), PSUM + bias f32,
tanh on ACT exact. Measured end-to-end rel err vs the fp32 reference:
~2e-4.
"""
import sys
import types

import numpy as np

B, N, D, CDIM = 32, 100000, 3, 128
NCORES = 8
BPC = B // NCORES          # batches per core
G = 10                     # point-groups per (batch, comp)
L = 10000                  # points per partition stream (N / G, exact)
P = 128                    # partitions
MM_F = 512                 # matmul free chunk (1 PSUM f32 bank)

CHUNKS = [1024, 2048, 2048, 2048, 2048, 784]   # sum == L
GROUPS = ((0, 1, 2), (3, 4, 5))
WSETS = (0, 1, 2, 2, 2, 2)

PROFILE = False            # set by test harness; collects HW exec time
LAST_EXEC_NS = None

_CACHE = {}


def _install_profile_shim():
    """Register the NTFF profile hook (missing antenv.axon_hooks in this
    container) so run_bass_kernel_spmd(trace=True) can report exec time."""
    if "antenv.axon_hooks" in sys.modules:
        return
    mod = types.ModuleType("antenv.axon_hooks")
    _state = {"hook": None}
    mod.set_axon_ntff_profile_hook = lambda h: _state.__setitem__("hook", h)
    mod.get_axon_ntff_profile_hook = lambda: _state["hook"]
    sys.modules["antenv.axon_hooks"] = mod
    try:
        from trn_agent_boot.trn_boot import _ntff_profile_via_ctypes
        mod.set_axon_ntff_profile_hook(
            _ntff_profile_via_ctypes("/opt/axon/libaxon_pjrt.so"))
    except Exception:
        pass
    import concourse.bass_utils as bu
    bu.upload_artifacts = lambda tmpdir: f"local:{tmpdir}"


def _build_program():
    import concourse.bass as bass
    import concourse.bacc as bacc
    from concourse import mybir

    f32 = mybir.dt.float32
    f16 = mybir.dt.float16
    Tanh = mybir.ActivationFunctionType.Tanh

    nc = bacc.Bacc("TRN2", target_bir_lowering=False, debug=False)
    x_d = nc.declare_dram_parameter("x", [P, L], f16, isOutput=False)
    w_d = nc.declare_dram_parameter("w", [P, 3 * P], f16, isOutput=False)
    d_d = nc.declare_dram_parameter("d", [P, 3], f32, isOutput=False)
    y_d = nc.declare_dram_parameter("y", [P, L], f16, isOutput=True)

    offs = [sum(CHUNKS[:i]) for i in range(len(CHUNKS))]
    NCHUNK = len(CHUNKS)

    # static SBUF tensors
    xin = [nc.alloc_sbuf_tensor(f"xin{c}", [P, CHUNKS[c]], f16).ap()
           for c in range(NCHUNK)]
    ha = [nc.alloc_sbuf_tensor(f"ha{c}", [P, CHUNKS[c]], f16).ap()
          for c in range(NCHUNK)]
    hb = [nc.alloc_sbuf_tensor(f"hb{c}", [P, CHUNKS[c]], f16).ap()
          for c in range(NCHUNK)]
    yout = [nc.alloc_sbuf_tensor(f"yout{c}", [P, CHUNKS[c]], f16).ap()
            for c in range(NCHUNK)]
    wall = nc.alloc_sbuf_tensor("wall", [P, 3 * P], f16).ap()
    biast = nc.alloc_sbuf_tensor("biast", [P, 3], f32).ap()
    ps = [nc.alloc_psum_tensor(f"ps{i}", [P, 2048], f32).ap()
          for i in range(2)]

    # stage enumeration: within a group, rotate over its 3 chunks so a
    # stage's input (ACT stage s-3) and PSUM WAR (ACT stage s-2) are both
    # covered by a single act_sem >= s-1 wait.
    stages = []
    for grp in GROUPS:
        for kblk in range(6):
            for ci in grp:
                stages.append((kblk, ci))
    idx = {kc: s for s, kc in enumerate(stages)}
    NST = len(stages)
    LAST = stages[-1]
    # act_sem value once stage s's tanh is complete (last stage = 2 halves)
    sz_last = CHUNKS[LAST[1]]
    half = (sz_last // 2 + 7) & ~7

    def h_in(kblk, ci):
        if kblk == 0:
            return xin[ci]
        return ha[ci] if kblk % 2 == 1 else hb[ci]

    def h_out(kblk, ci):
        if kblk == 5:
            return yout[ci]
        return ha[ci] if kblk % 2 == 0 else hb[ci]

    with (
        nc.Block() as block,
        nc.semaphore("dma_x0") as dx0,
        nc.semaphore("dma_x1") as dx1,
        nc.semaphore("dma_x2") as dx2,
        nc.semaphore("dma_x3") as dx3,
        nc.semaphore("dma_x4") as dx4,
        nc.semaphore("dma_x5") as dx5,
        nc.semaphore("dma_wd") as dwd,
        nc.semaphore("dma_out") as dout,
        nc.semaphore("act_sem") as act_sem,
        nc.semaphore("pe_sem") as pe_sem,
    ):
        dxs = [dx0, dx1, dx2, dx3, dx4, dx5]

        @block.gpsimd
        def _(g: bass.BassEngine):
            g.dma_start(out=wall, in_=w_d[:]).then_inc(dwd, 16)
            g.dma_start(out=biast, in_=d_d[:]).then_inc(dwd, 16)

        @block.sync
        def _(sync: bass.BassEngine):
            for c in range(NCHUNK):
                if c >= 3:
                    # bound in-flight HWDGE transfers (ring capacity)
                    sync.wait_ge(dxs[c - 2], 16)
                sync.dma_start(out=xin[c],
                               in_=x_d[:, offs[c]:offs[c] + CHUNKS[c]]
                               ).then_inc(dxs[c], 16)
            ndma = 0
            for ci in range(NCHUNK):
                c0, sz = offs[ci], CHUNKS[ci]
                if (5, ci) == LAST:
                    for a, b_, req in ((0, half, NST),
                                       (half, sz, NST + 1)):
                        sync.dma_start(out=y_d[:, c0 + a:c0 + b_],
                                       in_=yout[ci][:, a:b_]
                                       )._wait_ge(act_sem, req
                                                  ).then_inc(dout, 16)
                        ndma += 1
                else:
                    sync.dma_start(out=y_d[:, c0:c0 + sz], in_=yout[ci]
                                   )._wait_ge(act_sem, idx[(5, ci)] + 1
                                              ).then_inc(dout, 16)
                    ndma += 1
            sync.wait_ge(dout, 16 * ndma)

        @block.scalar
        def _(scalar: bass.BassEngine):
            for s, (kblk, ci) in enumerate(stages):
                ks = WSETS[kblk]
                sz = CHUNKS[ci]
                out_ap = h_out(kblk, ci)
                if (kblk, ci) == LAST:
                    # split the final tanh so the output DMA overlaps the
                    # second half instead of sitting exposed in the tail
                    for a, b_ in ((0, half), (half, sz)):
                        act = scalar.activation(
                            out_ap[:, a:b_], ps[s % 2][:, a:b_], Tanh,
                            bias=biast[:, ks:ks + 1], scale=1.0)
                        if a == 0:
                            act._wait_ge(pe_sem, s + 1)
                        act.then_inc(act_sem, 1)
                else:
                    scalar.activation(
                        out_ap, ps[s % 2][:, 0:sz], Tanh,
                        bias=biast[:, ks:ks + 1], scale=1.0,
                    )._wait_ge(pe_sem, s + 1).then_inc(act_sem, 1)

        @block.tensor
        def _(tensor: bass.BassEngine):
            tensor.wait_ge(dwd, 32)
            for s, (kblk, ci) in enumerate(stages):
                ks = WSETS[kblk]
                sz = CHUNKS[ci]
                if kblk == 0:
                    tensor.wait_ge(dxs[ci], 16)
                rhs = h_in(kblk, ci)
                for j in range(0, sz, MM_F):
                    je = min(j + MM_F, sz)
                    mm = tensor.matmul(ps[s % 2][:, j:je],
                                       wall[:, ks * P:(ks + 1) * P],
                                       rhs[:, j:je],
                                       start=True, stop=True)
                    if j == 0 and s >= 2:
                        # input-ready (s-3) + psum WAR (s-2), folded
                        mm._wait_ge(act_sem, s - 1)
                mm.then_inc(pe_sem, 1)

    nc.compile()
    return nc


def _film_params(c, Wk, bk, Wsk, bsk, Wbk, bbk):
    """A[b] = diag(scale[b]) @ Wk ; d[b] = scale[b]*bk + shift[b], float64."""
    c = c.astype(np.float64)
    scale = 1.0 / (1.0 + np.exp(-(c @ Wsk.astype(np.float64).T
                                  + bsk.astype(np.float64))))     # [B,3]
    shift = c @ Wbk.astype(np.float64).T + bbk.astype(np.float64)  # [B,3]
    A = scale[:, :, None] * Wk.astype(np.float64)[None]            # [B,3,3]
    d = scale * bk.astype(np.float64) + shift                      # [B,3]
    return A, d


def kernel(t, x, c,
           W0, b0, Ws0, bs0, Wb0, bb0,
           W1, b1, Ws1, bs1, Wb1, bb1,
           W2, b2, Ws2, bs2, Wb2, bb2):
    global LAST_EXEC_NS
    if PROFILE:
        _install_profile_shim()
    from concourse.bass_utils import run_bass_kernel_spmd

    x = np.asarray(x)
    c = np.asarray(c)
    (W0, b0, Ws0, bs0, Wb0, bb0, W1, b1, Ws1, bs1, Wb1, bb1,
     W2, b2, Ws2, bs2, Wb2, bb2) = (
        np.asarray(a) for a in (W0, b0, Ws0, bs0, Wb0, bb0,
                                W1, b1, Ws1, bs1, Wb1, bb1,
                                W2, b2, Ws2, bs2, Wb2, bb2))
    out_dtype = x.dtype

    if "prog" not in _CACHE:
        _CACHE["prog"] = _build_program()
    nc = _CACHE["prog"]

    # ---- host: FiLM affine params per (weight-set, batch), float64 ----
    sets = [
        _film_params(c, W0, b0, Ws0, bs0, Wb0, bb0),
        _film_params(c, W1, b1, Ws1, bs1, Wb1, bb1),
        _film_params(c, W2, b2, Ws2, bs2, Wb2, bb2),
    ]

    # ---- host: shard + relayout x ----
    # [B, N, 3] -> per core [128, L] fp16: p = b*32 + comp*10 + g
    xp = np.ascontiguousarray(x, dtype=np.float32)
    # [B, 3, G, L]
    xt = np.ascontiguousarray(xp.transpose(0, 2, 1)).reshape(B, D, G, L)

    in_maps = []
    for cc in range(NCORES):
        bs = range(cc * BPC, (cc + 1) * BPC)
        X = np.zeros((BPC, 32, L), np.float16)
        for i, b in enumerate(bs):
            X[i, :30] = xt[b].reshape(30, L)
        W6 = np.zeros((P, 3 * P), np.float16)
        D128 = np.zeros((P, 3), np.float32)
        for k in range(3):
            A, dv = sets[k]
            for i, b in enumerate(bs):
                for ci_ in range(3):
                    for cj in range(3):
                        a = np.float16(A[b, ci_, cj])
                        for g in range(G):
                            W6[i * 32 + cj * G + g,
                               k * P + i * 32 + ci_ * G + g] = a
                    D128[i * 32 + ci_ * G:i * 32 + ci_ * G + G, k] = \
                        np.float32(dv[b, ci_])
        in_maps.append({"x": X.reshape(P, L), "w": W6, "d": D128})

    res = run_bass_kernel_spmd(nc, in_maps, list(range(NCORES)),
                               trace=bool(PROFILE))
    if PROFILE:
        LAST_EXEC_NS = res.exec_time_ns

    # ---- host: gather + inverse layout ----
    out = np.empty((B, N, D), out_dtype)
    for cc in range(NCORES):
        Y = res.results[cc]["y"].reshape(BPC, 32, L)
        for i in range(BPC):
            b = cc * BPC + i
            # [30, L] -> [3, N] -> [N, 3]
            yb = Y[i, :30].reshape(D, N)
            out[b] = yb.T.astype(out_dtype, copy=False)
    return out


# revision 6
# speedup vs baseline: 1.0932x; 1.0932x over previous
"""Trainium2 Bass kernel for nn_FCond (FiLM-conditioned MLP chain).

Reference computation (B=32, N=100000, D=3, CDIM=128):
    h = x
    for kblk in [0, 1, 2, 2, 2, 2]:
        h = tanh((h @ Wk.T + bk) * sigmoid(c @ Wsk.T + bsk) + (c @ Wbk.T + bbk))

Since the FiLM conditioning depends only on (c, weights), each (batch,
block) reduces to an affine map  h' = tanh(A_kb @ h + d_kb)  with
A_kb [3,3], d_kb [3] precomputed on the host in float64.

Device strategy (pure data parallel over 8 cores, 4 batches/core):
  - Layout: partition p = b*32 + comp*10 + g  (4 batch-bands of 32
    partitions; 3 comps x 10 point-groups per band). Row 30 of each
    band is a constant-1.0 row, row 31 zero padding.
  - Each block is ONE block-diagonal [128x128] fp16 matmul on TensorE
    (40 real points per column), PSUM f32, then ScalarE does
    tanh(psum), evacuating PSUM->SBUF as fp16.
  - The affine bias d rides inside the matmul: weight column p gets
    d[p] in the ones-row, and the ones-row regenerates itself through
    every block via W[ones,ones]=16 (tanh(16) == 1.0 in fp16). No bias
    DMA, no per-partition bias operand in the activation.
  - Hand-scheduled engine programs (no TileContext), 30 stages
    s=(kblk, chunk) over 5 UNIFORM 2000-column chunks rotating in
    groups (0,1,2)/(3,4): uniform sizes keep the PE (1 cyc/col at its
    sustained 1.2 GHz p-state) and ACT (1 elem/cyc/lane, 1.2 GHz) in
    lockstep with ~100ns/stage of PE margin, so ACT — the roofline
    engine — never stalls. PSUM ping-pongs 2x[128,2048] (4 banks
    each); a single act_sem >= s-1 wait on the PE covers both the
    input dependency (ACT stage s-3 or s-2) and the PSUM WAR (s-2).
  - DMA: per-engine DGE rings are only ~125 GB/s, so transfers spread
    across the sync, vector and gpsimd queues (x chunk 0 split in
    half across two rings to cut the pipeline-fill latency). Outputs
    stream out per chunk as soon as the last block's tanh lands; the
    final chunk is split in half across two rings to hide the tail.

Numerics: weights/bias/activations fp16 (PE @ 1 cyc/col), PSUM f32,
tanh on ACT exact. Measured end-to-end rel err vs the fp32 reference:
~4e-4.
"""
import sys
import types

import numpy as np

B, N, D, CDIM = 32, 100000, 3, 128
NCORES = 8
BPC = B // NCORES          # batches per core
G = 10                     # point-groups per (batch, comp)
L = 10000                  # points per partition stream (N / G, exact)
P = 128                    # partitions
MM_F = 512                 # matmul free chunk (1 PSUM f32 bank)

CHUNK = 2000               # uniform column chunk
NCHUNK = 5
GROUPS = ((0, 1, 2), (3, 4))
WSETS = (0, 1, 2, 2, 2, 2)
X0SPLIT = 1024             # first chunk's DMA split point (2 rings)

PROFILE = False            # set by test harness; collects HW exec time
LAST_EXEC_NS = None

_CACHE = {}


def _install_profile_shim():
    """Register the NTFF profile hook (missing antenv.axon_hooks in this
    container) so run_bass_kernel_spmd(trace=True) can report exec time."""
    if "antenv.axon_hooks" in sys.modules:
        return
    mod = types.ModuleType("antenv.axon_hooks")
    _state = {"hook": None}
    mod.set_axon_ntff_profile_hook = lambda h: _state.__setitem__("hook", h)
    mod.get_axon_ntff_profile_hook = lambda: _state["hook"]
    sys.modules["antenv.axon_hooks"] = mod
    try:
        from trn_agent_boot.trn_boot import _ntff_profile_via_ctypes
        mod.set_axon_ntff_profile_hook(
            _ntff_profile_via_ctypes("/opt/axon/libaxon_pjrt.so"))
    except Exception:
        pass
    import concourse.bass_utils as bu
    bu.upload_artifacts = lambda tmpdir: f"local:{tmpdir}"


def _build_program():
    import concourse.bass as bass
    import concourse.bacc as bacc
    from concourse import mybir

    f32 = mybir.dt.float32
    f16 = mybir.dt.float16
    Tanh = mybir.ActivationFunctionType.Tanh

    nc = bacc.Bacc("TRN2", target_bir_lowering=False, debug=False)
    x_d = nc.declare_dram_parameter("x", [P, L], f16, isOutput=False)
    w_d = nc.declare_dram_parameter("w", [P, 3 * P], f16, isOutput=False)
    y_d = nc.declare_dram_parameter("y", [P, L], f16, isOutput=True)

    offs = [CHUNK * i for i in range(NCHUNK)]

    # static SBUF tensors
    xin = [nc.alloc_sbuf_tensor(f"xin{c}", [P, CHUNK], f16).ap()
           for c in range(NCHUNK)]
    ha = [nc.alloc_sbuf_tensor(f"ha{c}", [P, CHUNK], f16).ap()
          for c in range(NCHUNK)]
    hb = [nc.alloc_sbuf_tensor(f"hb{c}", [P, CHUNK], f16).ap()
          for c in range(NCHUNK)]
    yout = [nc.alloc_sbuf_tensor(f"yout{c}", [P, CHUNK], f16).ap()
            for c in range(NCHUNK)]
    wall = nc.alloc_sbuf_tensor("wall", [P, 3 * P], f16).ap()
    ps = [nc.alloc_psum_tensor(f"ps{i}", [P, 2048], f32).ap()
          for i in range(2)]

    # stage enumeration: rotation inside each group gives every stage's
    # input (ACT stage s-3 / s-2) and PSUM WAR (ACT s-2) a single
    # act_sem >= s-1 wait.
    stages = []
    for grp in GROUPS:
        for kblk in range(6):
            for ci in grp:
                stages.append((kblk, ci))
    idx = {kc: s for s, kc in enumerate(stages)}
    NST = len(stages)
    LAST = stages[-1]
    half = (CHUNK // 2 + 7) & ~7

    def h_in(kblk, ci):
        if kblk == 0:
            return xin[ci]
        return ha[ci] if kblk % 2 == 1 else hb[ci]

    def h_out(kblk, ci):
        if kblk == 5:
            return yout[ci]
        return ha[ci] if kblk % 2 == 0 else hb[ci]

    with (
        nc.Block() as block,
        nc.semaphore("dma_x0") as dx0,
        nc.semaphore("dma_x1") as dx1,
        nc.semaphore("dma_x2") as dx2,
        nc.semaphore("dma_x3") as dx3,
        nc.semaphore("dma_x4") as dx4,
        nc.semaphore("dma_w") as dww,
        nc.semaphore("dma_out") as dout,
        nc.semaphore("act_sem") as act_sem,
        nc.semaphore("pe_sem") as pe_sem,
    ):
        dxs = [dx0, dx1, dx2, dx3, dx4]

        def y_dma(eng, ci, a, b_, req):
            eng.dma_start(out=y_d[:, offs[ci] + a:offs[ci] + b_],
                          in_=yout[ci][:, a:b_]
                          )._wait_ge(act_sem, req).then_inc(dout, 16)

        # Per-engine DGE rings are ~125 GB/s each; spread transfers over
        # the engines allowed to issue DMAs (sync, gpsimd, scalar).
        @block.gpsimd
        def _(g: bass.BassEngine):
            g.dma_start(out=wall, in_=w_d[:]).then_inc(dww, 16)
            g.dma_start(out=xin[2], in_=x_d[:, offs[2]:offs[2] + CHUNK]
                        ).then_inc(dxs[2], 16)
            g.dma_start(out=xin[4], in_=x_d[:, offs[4]:offs[4] + CHUNK]
                        ).then_inc(dxs[4], 16)
            y_dma(g, 0, 0, CHUNK, idx[(5, 0)] + 1)
            y_dma(g, 3, 0, CHUNK, idx[(5, 3)] + 1)
            y_dma(g, 4, 0, half, NST)

        @block.sync
        def _(sync: bass.BassEngine):
            sync.dma_start(out=xin[0][:, 0:X0SPLIT],
                           in_=x_d[:, 0:X0SPLIT]).then_inc(dxs[0], 16)
            sync.dma_start(out=xin[1], in_=x_d[:, offs[1]:offs[1] + CHUNK]
                           ).then_inc(dxs[1], 16)
            sync.dma_start(out=xin[3], in_=x_d[:, offs[3]:offs[3] + CHUNK]
                           ).then_inc(dxs[3], 16)
            y_dma(sync, 1, 0, CHUNK, idx[(5, 1)] + 1)
            y_dma(sync, 2, 0, CHUNK, idx[(5, 2)] + 1)
            y_dma(sync, 4, half, CHUNK, NST + 1)
            sync.wait_ge(dout, 16 * 6)

        @block.scalar
        def _(scalar: bass.BassEngine):
            # second half of chunk 0 on the otherwise-idle ACT queue: the
            # issue cost lands during pipeline fill, and splitting chunk 0
            # across two rings halves the time to the first matmul
            scalar.dma_start(out=xin[0][:, X0SPLIT:],
                             in_=x_d[:, X0SPLIT:CHUNK]).then_inc(dxs[0], 16)
            for s, (kblk, ci) in enumerate(stages):
                if (kblk, ci) == LAST:
                    # split the final tanh so the two output DMAs overlap
                    # the remaining work instead of sitting in the tail
                    for a, b_ in ((0, half), (half, CHUNK)):
                        act = scalar.activation(
                            h_out(kblk, ci)[:, a:b_], ps[s % 2][:, a:b_],
                            Tanh, bias=0.0, scale=1.0)
                        if a == 0:
                            act._wait_ge(pe_sem, s + 1)
                        act.then_inc(act_sem, 1)
                else:
                    scalar.activation(
                        h_out(kblk, ci), ps[s % 2][:, 0:CHUNK], Tanh,
                        bias=0.0, scale=1.0,
                    )._wait_ge(pe_sem, s + 1).then_inc(act_sem, 1)

        @block.tensor
        def _(tensor: bass.BassEngine):
            tensor.wait_ge(dww, 16)
            for s, (kblk, ci) in enumerate(stages):
                ks = WSETS[kblk]
                if kblk == 0:
                    tensor.wait_ge(dxs[ci], 32 if ci == 0 else 16)
                rhs = h_in(kblk, ci)
                for j in range(0, CHUNK, MM_F):
                    je = min(j + MM_F, CHUNK)
                    mm = tensor.matmul(ps[s % 2][:, j:je],
                                       wall[:, ks * P:(ks + 1) * P],
                                       rhs[:, j:je],
                                       start=True, stop=True)
                    if j == 0 and s >= 2:
                        # input-ready + psum WAR, folded into one wait
                        mm._wait_ge(act_sem, s - 1)
                mm.then_inc(pe_sem, 1)

    nc.compile()
    return nc


def _film_params(c, Wk, bk, Wsk, bsk, Wbk, bbk):
    """A[b] = diag(scale[b]) @ Wk ; d[b] = scale[b]*bk + shift[b], float64."""
    c = c.astype(np.float64)
    scale = 1.0 / (1.0 + np.exp(-(c @ Wsk.astype(np.float64).T
                                  + bsk.astype(np.float64))))     # [B,3]
    shift = c @ Wbk.astype(np.float64).T + bbk.astype(np.float64)  # [B,3]
    A = scale[:, :, None] * Wk.astype(np.float64)[None]            # [B,3,3]
    d = scale * bk.astype(np.float64) + shift                      # [B,3]
    return A, d


def kernel(t, x, c,
           W0, b0, Ws0, bs0, Wb0, bb0,
           W1, b1, Ws1, bs1, Wb1, bb1,
           W2, b2, Ws2, bs2, Wb2, bb2):
    global LAST_EXEC_NS
    if PROFILE:
        _install_profile_shim()
    from concourse.bass_utils import run_bass_kernel_spmd

    x = np.asarray(x)
    c = np.asarray(c)
    (W0, b0, Ws0, bs0, Wb0, bb0, W1, b1, Ws1, bs1, Wb1, bb1,
     W2, b2, Ws2, bs2, Wb2, bb2) = (
        np.asarray(a) for a in (W0, b0, Ws0, bs0, Wb0, bb0,
                                W1, b1, Ws1, bs1, Wb1, bb1,
                                W2, b2, Ws2, bs2, Wb2, bb2))
    out_dtype = x.dtype

    if "prog" not in _CACHE:
        _CACHE["prog"] = _build_program()
    nc = _CACHE["prog"]

    # ---- host: FiLM affine params per (weight-set, batch), float64 ----
    sets = [
        _film_params(c, W0, b0, Ws0, bs0, Wb0, bb0),
        _film_params(c, W1, b1, Ws1, bs1, Wb1, bb1),
        _film_params(c, W2, b2, Ws2, bs2, Wb2, bb2),
    ]

    # ---- host: shard + relayout x ----
    # [B, N, 3] -> per core [128, L] fp16: p = b*32 + comp*10 + g
    xp = np.ascontiguousarray(x, dtype=np.float32)
    # [B, 3, G, L]
    xt = np.ascontiguousarray(xp.transpose(0, 2, 1)).reshape(B, D, G, L)

    in_maps = []
    for cc in range(NCORES):
        bs = range(cc * BPC, (cc + 1) * BPC)
        X = np.zeros((BPC, 32, L), np.float16)
        for i, b in enumerate(bs):
            X[i, :30] = xt[b].reshape(30, L)
            X[i, 30] = 1.0          # ones-row: carries the bias via matmul
        W6 = np.zeros((P, 3 * P), np.float16)
        for k in range(3):
            A, dv = sets[k]
            for i, b in enumerate(bs):
                ones_r = i * 32 + 30
                for ci_ in range(3):
                    for cj in range(3):
                        a = np.float16(A[b, ci_, cj])
                        for g in range(G):
                            W6[i * 32 + cj * G + g,
                               k * P + i * 32 + ci_ * G + g] = a
                    # bias d rides the ones-row
                    W6[ones_r, k * P + i * 32 + ci_ * G:
                       k * P + i * 32 + ci_ * G + G] = np.float16(dv[b, ci_])
                # ones-row regenerates itself: tanh(16.0) == 1.0 in fp16
                W6[ones_r, k * P + ones_r] = np.float16(16.0)
        in_maps.append({"x": X.reshape(P, L), "w": W6})

    res = run_bass_kernel_spmd(nc, in_maps, list(range(NCORES)),
                               trace=bool(PROFILE))
    if PROFILE:
        LAST_EXEC_NS = res.exec_time_ns

    # ---- host: gather + inverse layout ----
    out = np.empty((B, N, D), out_dtype)
    for cc in range(NCORES):
        Y = res.results[cc]["y"].reshape(BPC, 32, L)
        for i in range(BPC):
            b = cc * BPC + i
            # [30, L] -> [3, N] -> [N, 3]
            yb = Y[i, :30].reshape(D, N)
            out[b] = yb.T.astype(out_dtype, copy=False)
    return out
